# revision 5
# baseline (speedup 1.0000x reference)
"""Trainium2 Bass kernel for DeiT self-attention with channel-pruning masks.

Reference computation (B=16, S=577, HID=768, H=12, D=64, N_KEEP=576):
    q/k/v = hs @ W + b            [B,S,576]
    scatter channels to [B,S,768] at {q,k,v}_idx, split into 12 heads of 64
    softmax attention per (b, h), concat heads, gather v_idx channels.

Strategy:
  - The channel scatters are folded into the weight matrices on the host
    (zero columns at dropped channels), so the device kernel is a dense
    attention over the full 768-channel layout.
  - Data-parallel over batch: 8 cores x 2 images each.
  - Per core the device computes (token count T = 2*577 = 1154):
      Q^T, K^T = W^T-stationary matmuls           [768 ch, T]    (SBUF)
      V_aug    = hsT-stationary matmul            [T, 780]       (SBUF)
                 (per head: 64 value cols + one bias-column of ones)
      per (image, head):  S^T = K_h^T x Q_h       [ktok, qtok] PSUM
                          E = exp(S^T / 8)        (ScalarE, from PSUM)
                          ctxU^T|Z = [V_h|1]^T x E  accumulate over ktok
                          ctx^T = ctxU^T * (1/Z)  (DVE, Z broadcast via DRAM)
      output written as ctx^T [768, T]; host transposes + gathers v_idx.
  - Matmuls run as float32r (full fp32 data, 4x streaming rate vs plain
    fp32 when the moving dim is >= 256).
"""

import numpy as np

B, S, HID = 16, 577, 768
H, D = 12, 64
N_KEEP = 576
NCORES = 8
BPC = B // NCORES          # images per core
TOK = BPC * S              # tokens per core
VW = H * (D + 1)           # 780: V columns augmented with per-head ones column
P = 128
ICH = HID // P             # 6 input-channel chunks
OCH = HID // P             # 6 q/k output-channel chunks
TOK_TILES = [(0, 386), (386, 386), (772, 382)]      # projection moving tiles (even)
KCHUNKS = [(0, 128), (128, 128), (256, 128), (384, 128), (512, 65)]  # per image
QTILES = [(0, 290), (287, 290)]                     # per image score tiles (even, 3-col overlap)

_NC_CACHE = {}


def _build_nc(use_f32r=True):
    import concourse.bacc as bacc
    import concourse.mybir as mybir
    import concourse.tile as tile

    f32 = mybir.dt.float32
    mm_dt = mybir.dt.float32r if use_f32r else mybir.dt.float32

    nc = bacc.Bacc("TRN2", target_bir_lowering=False)

    hsT = nc.dram_tensor("hsT", [HID, TOK], mm_dt, kind="ExternalInput")
    wq = nc.dram_tensor("wq", [HID, HID], mm_dt, kind="ExternalInput")
    wk = nc.dram_tensor("wk", [HID, HID], mm_dt, kind="ExternalInput")
    wv = nc.dram_tensor("wv", [HID, VW], mm_dt, kind="ExternalInput")
    bq = nc.dram_tensor("bq", [HID], f32, kind="ExternalInput")
    bk = nc.dram_tensor("bk", [HID], f32, kind="ExternalInput")
    bvb = nc.dram_tensor("bvb", [P, VW], f32, kind="ExternalInput")
    outT = nc.dram_tensor("outT", [HID, TOK], f32, kind="ExternalOutput")

    def mm(out_ps, lhsT, rhs, start, stop):
        nc.tensor.matmul(out_ps, lhsT, rhs, start=start, stop=stop)

    with tile.TileContext(nc) as tc:
        with (
            tc.tile_pool(name="big", bufs=1) as big,
            tc.tile_pool(name="ps", bufs=8, space="PSUM") as ps,
            tc.tile_pool(name="dram", bufs=4, space="DRAM") as dpool,
        ):
            # ---- persistent SBUF tensors ----
            wv_sb = big.tile([P, ICH, VW], mm_dt)
            nc.sync.dma_start(wv_sb[:], wv.rearrange("(c p) n -> p c n", p=P))
            bvb_sb = big.tile([P, VW], f32)
            nc.sync.dma_start(bvb_sb[:], bvb[:])
            bq_sb = big.tile([P, OCH], f32)
            nc.sync.dma_start(bq_sb[:], bq.rearrange("(c p) -> p c", p=P))
            bk_sb = big.tile([P, OCH], f32)
            nc.sync.dma_start(bk_sb[:], bk.rearrange("(c p) -> p c", p=P))

            q_sb = big.tile([P, OCH, TOK], mm_dt)
            k_sb = big.tile([P, OCH, TOK], mm_dt)
            v_sb = big.tile([P, BPC * len(KCHUNKS), VW], mm_dt)

            # ---- phase 1: projections (hsT + Wq/Wk live only here) ----
            with tc.tile_pool(name="ph1", bufs=1) as ph1:
                hsT_sb = ph1.tile([P, ICH, TOK], mm_dt)
                nc.sync.dma_start(hsT_sb[:], hsT.rearrange("(c p) t -> p c t", p=P))
                wq_sb = ph1.tile([P, ICH, HID], mm_dt)
                nc.sync.dma_start(wq_sb[:], wq.rearrange("(c p) n -> p c n", p=P))
                wk_sb = ph1.tile([P, ICH, HID], mm_dt)
                nc.sync.dma_start(wk_sb[:], wk.rearrange("(c p) n -> p c n", p=P))

                # V natural [tok, ch]: hsT chunks stationary, Wv moving
                VT = VW // 2  # 390, head-aligned (6 heads x 65)
                for b in range(BPC):
                    for j, (koff, kcs) in enumerate(KCHUNKS):
                        toff = b * S + koff
                        for n in range(2):
                            vp = ps.tile([P, 512], f32, tag="ps", name="vp")[:kcs, :VT]
                            for k in range(ICH):
                                mm(
                                    vp,
                                    hsT_sb[:, k, toff : toff + kcs],
                                    wv_sb[:, k, n * VT : (n + 1) * VT],
                                    start=(k == 0),
                                    stop=(k == ICH - 1),
                                )
                            nc.vector.tensor_add(
                                out=v_sb[:kcs, b * 5 + j, n * VT : (n + 1) * VT],
                                in0=vp,
                                in1=bvb_sb[:kcs, n * VT : (n + 1) * VT],
                            )

                # Q^T / K^T: W chunks stationary, hsT moving
                for w_sb, b_sb, dst in ((wq_sb, bq_sb, q_sb), (wk_sb, bk_sb, k_sb)):
                    for i in range(OCH):
                        qps = []
                        for toff, tcs in TOK_TILES:
                            qps.append(ps.tile([P, 512], f32, tag="ps", name="qp")[:, :tcs])
                        for k in range(ICH):
                            for t, (toff, tcs) in enumerate(TOK_TILES):
                                mm(
                                    qps[t],
                                    w_sb[:, k, i * P : (i + 1) * P],
                                    hsT_sb[:, k, toff : toff + tcs],
                                    start=(k == 0),
                                    stop=(k == ICH - 1),
                                )
                        for t, (toff, tcs) in enumerate(TOK_TILES):
                            nc.vector.tensor_add(
                                out=dst[:, i, toff : toff + tcs],
                                in0=qps[t],
                                in1=b_sb[:, i : i + 1].to_broadcast((P, tcs)),
                            )

            # ---- phase 2: attention per (image, head) ----
            with (
                tc.tile_pool(name="epool", bufs=2) as epool,
                tc.tile_pool(name="opool", bufs=4) as opool,
                tc.tile_pool(name="spool", bufs=4) as spool,
            ):
                Exp = mybir.ActivationFunctionType.Exp
                for b in range(BPC):
                    for h in range(H):
                        pb = 64 * (h % 2)
                        chk = h // 2
                        e_sb = epool.tile([P, len(KCHUNKS), S], mm_dt, tag="e")
                        for qo, qcs in QTILES:
                            # scores S^T, chunk by chunk over ktok
                            for c, (ko, kcs) in enumerate(KCHUNKS):
                                sp = ps.tile([P, 512], f32, tag="ps", name="sp")[:kcs, :qcs]
                                mm(
                                    sp,
                                    k_sb[pb : pb + 64, chk, b * S + ko : b * S + ko + kcs],
                                    q_sb[pb : pb + 64, chk, b * S + qo : b * S + qo + qcs],
                                    start=True,
                                    stop=True,
                                )
                                nc.scalar.activation(
                                    e_sb[:kcs, c, qo : qo + qcs],
                                    sp,
                                    Exp,
                                    scale=0.125,
                                )
                            # ctxU^T (rows 0:64) and Z (row 64), accumulated over ktok
                            cp = ps.tile([P, 512], f32, tag="ps", name="cp")[:65, :qcs]
                            for c, (ko, kcs) in enumerate(KCHUNKS):
                                mm(
                                    cp,
                                    v_sb[:kcs, b * 5 + c, h * 65 : (h + 1) * 65],
                                    e_sb[:kcs, c, qo : qo + qcs],
                                    start=(c == 0),
                                    stop=(c == len(KCHUNKS) - 1),
                                )
                            # normalize: broadcast 1/Z across 64 partitions via DRAM
                            zr = spool.tile([1, 512], f32, tag="zr", name="zr")[:, :qcs]
                            nc.vector.reciprocal(zr, cp[64:65, :qcs])
                            zd = dpool.tile([1, 512], f32)
                            nc.sync.dma_start(zd[0:1, :qcs], zr)
                            zb = spool.tile([64, 512], f32, tag="zb", name="zb")[:, :qcs]
                            nc.sync.dma_start(zb, zd[0:1, :qcs].to_broadcast((64, qcs)))
                            o_sb = opool.tile([64, 512], f32, tag="o", name="o_sb")[:, :qcs]
                            nc.vector.tensor_mul(out=o_sb, in0=cp[:64, :qcs], in1=zb)
                            nc.sync.dma_start(
                                outT[h * 64 : (h + 1) * 64, b * S + qo : b * S + qo + qcs],
                                o_sb,
                            )

    nc.compile()
    return nc


def _get_nc(use_f32r=True):
    key = ("nc", use_f32r)
    if key not in _NC_CACHE:
        _NC_CACHE[key] = _build_nc(use_f32r)
    return _NC_CACHE[key]


def _round_fp32r(x):
    """Round fp32 -> fp32r bit pattern (11-bit stored mantissa, RNE), fp32 container."""
    u = np.ascontiguousarray(x).view(np.uint32).astype(np.uint64)
    u = u + 0x7FF + ((u >> 12) & 1)
    return (u & ~np.uint64(0xFFF)).astype(np.uint32).view(np.float32).reshape(x.shape)


def _make_in_maps(hidden_states, Wq, bq, Wk, bk, Wv, bv, q_idx, k_idx, v_idx,
                  use_f32r=True):
    f32 = np.float32
    hs = np.asarray(hidden_states, f32)
    q_idx = np.asarray(q_idx).astype(np.int64)
    k_idx = np.asarray(k_idx).astype(np.int64)
    v_idx = np.asarray(v_idx).astype(np.int64)

    # fold channel scatters into full-width weights
    wq_full = np.zeros((HID, HID), f32)
    wq_full[:, q_idx] = np.asarray(Wq, f32)
    bq_full = np.zeros(HID, f32)
    bq_full[q_idx] = np.asarray(bq, f32)
    wk_full = np.zeros((HID, HID), f32)
    wk_full[:, k_idx] = np.asarray(Wk, f32)
    bk_full = np.zeros(HID, f32)
    bk_full[k_idx] = np.asarray(bk, f32)

    wv_full = np.zeros((HID, HID), f32)
    wv_full[:, v_idx] = np.asarray(Wv, f32)
    bv_full = np.zeros(HID, f32)
    bv_full[v_idx] = np.asarray(bv, f32)
    # augmented V layout: per head 64 value cols + a ones column (softmax denom)
    wv_aug = np.zeros((HID, VW), f32)
    bv_aug = np.zeros(VW, f32)
    for h in range(H):
        wv_aug[:, h * 65 : h * 65 + 64] = wv_full[:, h * 64 : (h + 1) * 64]
        bv_aug[h * 65 : h * 65 + 64] = bv_full[h * 64 : (h + 1) * 64]
        bv_aug[h * 65 + 64] = 1.0
    bvb = np.broadcast_to(bv_aug, (P, VW)).copy()

    if use_f32r:
        wq_full = _round_fp32r(wq_full)
        wk_full = _round_fp32r(wk_full)
        wv_aug = _round_fp32r(wv_aug)

    in_maps = []
    for c in range(NCORES):
        hsT = np.ascontiguousarray(
            hs[c * BPC : (c + 1) * BPC].reshape(TOK, HID).T
        )
        if use_f32r:
            hsT = _round_fp32r(hsT)
        in_maps.append(
            {
                "hsT": hsT,
                "wq": wq_full,
                "wk": wk_full,
                "wv": wv_aug,
                "bq": bq_full,
                "bk": bk_full,
                "bvb": bvb,
            }
        )
    return in_maps, v_idx


def _assemble_output(results, v_idx):
    ctx = np.empty((B, S, HID), np.float32)
    for c in range(NCORES):
        outT = results[c]["outT"]  # [HID, TOK]
        ctx[c * BPC : (c + 1) * BPC] = outT.T.reshape(BPC, S, HID)
    return np.ascontiguousarray(ctx[:, :, v_idx])


def run(inputs, trace=False, use_f32r=True, **spmd_kwargs):
    """Full pipeline; returns (output, BassKernelResults)."""
    from concourse import bass_utils

    in_maps, v_idx = _make_in_maps(**inputs, use_f32r=use_f32r)
    nc = _get_nc(use_f32r)
    res = bass_utils.run_bass_kernel_spmd(
        nc, in_maps, core_ids=list(range(NCORES)), trace=trace, **spmd_kwargs
    )
    return _assemble_output(res.results, v_idx), res


def kernel(**inputs):
    out, _ = run(inputs, trace=False)
    return out
